# revision 2
# baseline (speedup 1.0000x reference)
"""Trainium2 Bass kernel for AdaptiveReLULayer (MoE-style routed batched matmul).

    out[b] = LeakyReLU_0.2(x[b] @ weight[indices[b]] + bias)
    x: [2048, 256, 256] f32, indices: [2048] int, weight: [1024, 256, 256] f32

Strategy: pure data parallelism over the batch dim B=2048 across 8
NeuronCores (256 batches/core).  The per-batch weight gather is resolved on
the host while sharding (the gather is pure addressing — the device-side HBM
traffic is identical either way), and x is pre-transposed on the host into
the exact SBUF tile layout, so every device DMA is a single fully-contiguous
block transfer.  Compute runs in bf16 with fp32 PSUM accumulation
(~3e-3 relative error); LeakyReLU is one DVE op: (psum*0.2) max psum.
"""

import numpy as np
import ml_dtypes

import concourse.bass as bass
import concourse.tile as tile
import concourse.mybir as mybir
from concourse import bacc
from concourse.bass_utils import run_bass_kernel_spmd

B, NTOK, DIN, DOUT, C = 2048, 256, 256, 256, 1024
NCORES = 8
BLOC = B // NCORES          # 256 batches per core
G = 8                       # batches per tile-group (one DMA round)
NG = BLOC // G              # 32 groups per core
KC = DIN // 128             # contraction chunks of 128
TCH = NTOK // 128           # token chunks of 128
NEG_SLOPE = 0.2
BF16 = mybir.dt.bfloat16
F32 = mybir.dt.float32

LAST = {}                   # stash of the last run's BassKernelResults


def _build(nonzero_bias: bool):
    nc = bacc.Bacc(
        "TRN2", target_bir_lowering=False, debug=False, num_devices=NCORES
    )
    xt_d = nc.dram_tensor("xt", [NG, 128, G, KC, NTOK], BF16, kind="ExternalInput")
    wg_d = nc.dram_tensor("wg", [NG, 128, G, KC, DOUT], BF16, kind="ExternalInput")
    bias_d = (
        nc.dram_tensor("bias", [1, DOUT], F32, kind="ExternalInput")
        if nonzero_bias
        else None
    )
    out_d = nc.dram_tensor("out", [NG, 128, G, TCH, DOUT], BF16, kind="ExternalOutput")

    with tile.TileContext(nc) as tc:
        with (
            tc.tile_pool(name="io", bufs=3) as io,
            tc.tile_pool(name="psum", bufs=8, space=bass.MemorySpace.PSUM) as psum,
            tc.tile_pool(name="one", bufs=1) as one,
        ):
            bias_t = None
            if nonzero_bias:
                bias_t = one.tile([128, DOUT], F32, tag="bias")
                bap = bias_d.ap()
                nc.sync.dma_start(
                    out=bias_t[:],
                    in_=bass.AP(tensor=bap.tensor, offset=bap.offset,
                                ap=[[0, 128], bap.ap[1]]),
                )
            for g in range(NG):
                xt_t = io.tile([128, G, KC, NTOK], BF16, tag="xt")
                nc.sync.dma_start(out=xt_t[:], in_=xt_d[g])
                wg_t = io.tile([128, G, KC, DOUT], BF16, tag="wg")
                nc.sync.dma_start(out=wg_t[:], in_=wg_d[g])
                out_t = io.tile([128, G, TCH, DOUT], BF16, tag="out")
                for j in range(G):
                    for t in range(TCH):
                        ps = psum.tile([128, DOUT], F32, tag="ps")
                        for k in range(KC):
                            nc.tensor.matmul(
                                ps[:],
                                xt_t[:, j, k, t * 128 : (t + 1) * 128],
                                wg_t[:, j, k, :],
                                start=(k == 0),
                                stop=(k == KC - 1),
                            )
                        if nonzero_bias:
                            tmp = io.tile([128, DOUT], F32, tag="tmp")
                            nc.vector.scalar_tensor_tensor(
                                out=tmp[:], in0=ps[:], scalar=1.0, in1=bias_t[:],
                                op0=mybir.AluOpType.mult, op1=mybir.AluOpType.add,
                            )
                            nc.vector.scalar_tensor_tensor(
                                out=out_t[:, j, t, :], in0=tmp[:], scalar=NEG_SLOPE,
                                in1=tmp[:],
                                op0=mybir.AluOpType.mult, op1=mybir.AluOpType.max,
                            )
                        else:
                            # LeakyReLU = max(0.2*v, v); DVE may read only one
                            # PSUM input, so ACT first copies psum -> SBUF.
                            cp = io.tile([128, DOUT], BF16, tag="cp")
                            nc.scalar.activation(
                                out=cp[:], in_=ps[:],
                                func=mybir.ActivationFunctionType.Copy,
                            )
                            nc.vector.scalar_tensor_tensor(
                                out=out_t[:, j, t, :], in0=ps[:], scalar=NEG_SLOPE,
                                in1=cp[:],
                                op0=mybir.AluOpType.mult, op1=mybir.AluOpType.max,
                            )
                nc.scalar.dma_start(out=out_d[g], in_=out_t[:])
    nc.compile()
    return nc


def kernel(x, indices, weight, bias, _trace=False):
    x = np.asarray(x)
    indices = np.asarray(indices).astype(np.int64)
    weight = np.asarray(weight)
    bias = np.asarray(bias)

    # x[b, n, i] with b=(c,g,j), i=(k,p)  ->  xt[c, g, p, j, k, n]
    xb = x.astype(ml_dtypes.bfloat16)
    xt = np.ascontiguousarray(
        xb.reshape(NCORES, NG, G, NTOK, KC, 128).transpose(0, 1, 5, 2, 4, 3)
    )
    # weight[cls, i, o] with i=(k,p), gathered at indices -> wg[c, g, p, j, k, o]
    wb = weight.astype(ml_dtypes.bfloat16).reshape(C, KC, 128, DOUT)
    wg = np.ascontiguousarray(
        wb[indices]
        .reshape(NCORES, NG, G, KC, 128, DOUT)
        .transpose(0, 1, 4, 2, 3, 5)
    )

    nonzero_bias = bool(np.any(bias))
    nc = _build(nonzero_bias)

    in_maps = []
    for c in range(NCORES):
        m = {"xt": xt[c], "wg": wg[c]}
        if nonzero_bias:
            m["bias"] = np.ascontiguousarray(bias.reshape(1, DOUT).astype(np.float32))
        in_maps.append(m)

    res = run_bass_kernel_spmd(
        nc, in_maps, core_ids=list(range(NCORES)), trace=_trace
    )
    LAST["results"] = res

    # out[c][g, p, j, t, o] -> out[b=(c,g,j), n=(t,p), o]
    outs = []
    for c in range(NCORES):
        o = np.asarray(res.results[c]["out"])
        o = o.transpose(0, 2, 3, 1, 4).reshape(BLOC, NTOK, DOUT)
        outs.append(o)
    return np.concatenate(outs, axis=0).astype(np.float32)


# revision 6
# speedup vs baseline: 1.2718x; 1.2718x over previous
"""Trainium2 Bass kernel for AdaptiveReLULayer (MoE-style routed batched matmul).

    out[b] = LeakyReLU_0.2(x[b] @ weight[indices[b]] + bias)
    x: [2048, 256, 256] f32, indices: [2048] int, weight: [1024, 256, 256] f32

Strategy: data parallelism over the batch dim B=2048 across 8 NeuronCores
(256 batches/core), with an index-aware schedule: batches that share a weight
index are assigned to the same core as a "run" (length 1..4), so each run's
weight tile is DMA'd from HBM once and reused from SBUF (~47% less weight
traffic).  Run-length COUNTS are equalized across cores by splitting runs, so
all 8 cores execute the same static SPMD graph; only the data differs.

The weight gather itself is resolved on the host while sharding (pure
addressing), and x is pre-permuted/transposed on the host into the exact SBUF
tile layout, so every device DMA is one fully-contiguous 2 MiB block.
Compute is bf16 with fp32 PSUM accumulation (~3e-3 relative error).
LeakyReLU: ACT copies the 512-wide PSUM bank to SBUF (bf16), then one DVE
scalar_tensor_tensor computes (v*0.2) max v in place.
"""

import numpy as np
import ml_dtypes

import concourse.bass as bass
import concourse.tile as tile
import concourse.mybir as mybir
from concourse import bacc
from concourse.bass_utils import run_bass_kernel_spmd

B, NTOK, DIN, DOUT, C = 2048, 256, 256, 256, 1024
NCORES = 8
BLOC = B // NCORES          # 256 batches per core
G = 16                      # batches per x/out tile-group (one 2MiB DMA)
NG = BLOC // G              # 16 groups per core
GW = 16                     # weight runs per weight-group DMA (2MiB)
KC = DIN // 128             # contraction chunks of 128
TCH = NTOK // 128           # token chunks of 128
MAXRUN = 4
NEG_SLOPE = 0.2
BF16 = mybir.dt.bfloat16
F32 = mybir.dt.float32

LAST = {}                   # stash of the last run's BassKernelResults


def _schedule(indices):
    """Partition the 2048 batches into 8 cores of 256 as runs of equal-index
    batches (length 1..MAXRUN).  Returns (run_lengths, perm, wu_cls) where
    run_lengths is one shared list (identical structure for all cores), perm
    is [NCORES, BLOC] global batch ids in processing order, and wu_cls is
    [NCORES, NRUNS] the weight class per run.
    """
    by_cls = {}
    for b, c in enumerate(indices.tolist()):
        by_cls.setdefault(c, []).append(b)

    runs = []  # (class, [batch ids]) with len <= MAXRUN
    for c, bs in by_cls.items():
        for i in range(0, len(bs), MAXRUN):
            runs.append((c, bs[i : i + MAXRUN]))
    runs.sort(key=lambda r: -len(r[1]))

    # greedy bin-pack into cores of capacity BLOC, splitting when needed
    caps = [BLOC] * NCORES
    core_runs = [[] for _ in range(NCORES)]
    for c, bs in runs:
        while bs:
            k = int(np.argmax(caps))
            take = min(len(bs), caps[k])
            assert take > 0
            core_runs[k].append((c, bs[:take]))
            caps[k] -= take
            bs = bs[take:]
    assert all(v == 0 for v in caps)

    # equalize run-length counts across cores by splitting longer runs
    def counts(rl):
        n = [0] * (MAXRUN + 1)
        for c, bs in rl:
            n[len(bs)] += 1
        return n

    for L in range(MAXRUN, 1, -1):
        tgt = min(counts(rl)[L] for rl in core_runs)
        for rl in core_runs:
            while counts(rl)[L] > tgt:
                i = next(i for i, r in enumerate(rl) if len(r[1]) == L)
                c, bs = rl.pop(i)
                h = L // 2
                rl.append((c, bs[:h]))
                rl.append((c, bs[h:]))

    cn = counts(core_runs[0])
    assert all(counts(rl) == cn for rl in core_runs), [counts(rl) for rl in core_runs]

    # canonical order: longest runs first
    for rl in core_runs:
        rl.sort(key=lambda r: -len(r[1]))
    run_lengths = [len(bs) for c, bs in core_runs[0]]
    perm = np.array(
        [[b for c, bs in rl for b in bs] for rl in core_runs], dtype=np.int64
    )
    wu_cls = np.array([[c for c, bs in rl] for rl in core_runs], dtype=np.int64)
    return run_lengths, perm, wu_cls


def _build(run_lengths, nonzero_bias: bool):
    nruns = len(run_lengths)
    nwg = -(-nruns // GW)
    nc = bacc.Bacc(
        "TRN2", target_bir_lowering=False, debug=False, num_devices=NCORES
    )
    xt_d = nc.dram_tensor("xt", [NG, 128, G, KC, NTOK], BF16, kind="ExternalInput")
    wu_d = nc.dram_tensor("wu", [nwg, 128, GW, KC, DOUT], BF16, kind="ExternalInput")
    bias_d = (
        nc.dram_tensor("bias", [1, DOUT], F32, kind="ExternalInput")
        if nonzero_bias
        else None
    )
    out_d = nc.dram_tensor("out", [NG, 128, G, TCH, DOUT], BF16, kind="ExternalOutput")

    with tile.TileContext(nc) as tc:
        with (
            tc.tile_pool(name="io", bufs=3) as io,
            tc.tile_pool(name="wp", bufs=2) as wp,
            tc.tile_pool(name="psum", bufs=8, space=bass.MemorySpace.PSUM) as psum,
            tc.tile_pool(name="one", bufs=1) as one,
        ):
            bias_t = None
            if nonzero_bias:
                bias_t = one.tile([128, TCH, DOUT], F32, tag="bias")
                bap = bias_d.ap()
                nc.sync.dma_start(
                    out=bias_t[:],
                    in_=bass.AP(tensor=bap.tensor, offset=bap.offset,
                                ap=[[0, 128], [0, TCH], bap.ap[1]]),
                )

            xt_t = None
            out_t = None
            wu_t = None
            bp = 0  # batch position within this core, in permuted order
            for r, L in enumerate(run_lengths):
                sw = r % GW
                if sw == 0:
                    wu_t = wp.tile([128, GW, KC, DOUT], BF16, tag="wu")
                    nc.sync.dma_start(out=wu_t[:], in_=wu_d[r // GW])
                for i in range(L):
                    g, jj = divmod(bp + i, G)
                    if jj == 0:
                        if out_t is not None:
                            nc.scalar.dma_start(out=out_d[g - 1], in_=out_t[:])
                        xt_t = io.tile([128, G, KC, NTOK], BF16, tag="xt")
                        nc.sync.dma_start(out=xt_t[:], in_=xt_d[g])
                        out_t = io.tile([128, G, TCH, DOUT], BF16, tag="out")
                    ps = psum.tile([128, TCH, DOUT], F32, tag="ps")
                    for t in range(TCH):
                        for k in range(KC):
                            nc.tensor.matmul(
                                ps[:, t, :],
                                xt_t[:, jj, k, t * 128 : (t + 1) * 128],
                                wu_t[:, sw, k, :],
                                start=(k == 0),
                                stop=(k == KC - 1),
                            )
                    if nonzero_bias:
                        tmp = io.tile([128, TCH, DOUT], F32, tag="tmp")
                        nc.vector.scalar_tensor_tensor(
                            out=tmp[:], in0=ps[:, :, :], scalar=1.0,
                            in1=bias_t[:],
                            op0=mybir.AluOpType.mult, op1=mybir.AluOpType.add,
                        )
                        nc.vector.scalar_tensor_tensor(
                            out=out_t[:, jj, :, :], in0=tmp[:],
                            scalar=NEG_SLOPE, in1=tmp[:],
                            op0=mybir.AluOpType.mult, op1=mybir.AluOpType.max,
                        )
                    else:
                        # ACT: 512-wide PSUM bank -> SBUF bf16 copy
                        nc.scalar.activation(
                            out=out_t[:, jj, :, :], in_=ps[:, :, :],
                            func=mybir.ActivationFunctionType.Copy,
                        )
                        # DVE: in-place LeakyReLU  v = (v*0.2) max v
                        nc.vector.scalar_tensor_tensor(
                            out=out_t[:, jj, :, :], in0=out_t[:, jj, :, :],
                            scalar=NEG_SLOPE, in1=out_t[:, jj, :, :],
                            op0=mybir.AluOpType.mult, op1=mybir.AluOpType.max,
                        )
                bp += L
            assert bp == BLOC
            nc.scalar.dma_start(out=out_d[NG - 1], in_=out_t[:])
    nc.compile()
    return nc


def kernel(x, indices, weight, bias, _trace=False):
    x = np.asarray(x)
    indices = np.asarray(indices).astype(np.int64)
    weight = np.asarray(weight)
    bias = np.asarray(bias)

    run_lengths, perm, wu_cls = _schedule(indices)
    nruns = len(run_lengths)
    nwg = -(-nruns // GW)

    # x[b, n, i] at permuted b, with i=(k,p)  ->  xt[c, g, p, j, k, n]
    xb = x.astype(ml_dtypes.bfloat16)
    xt = np.ascontiguousarray(
        xb[perm.reshape(-1)]
        .reshape(NCORES, NG, G, NTOK, KC, 128)
        .transpose(0, 1, 5, 2, 4, 3)
    )
    # weight[cls, i, o] with i=(k,p) at the per-run classes -> wu[c, wg, p, s, k, o]
    wb = weight.astype(ml_dtypes.bfloat16).reshape(C, KC, 128, DOUT)
    pad = nwg * GW - nruns
    wu_cls_p = np.concatenate(
        [wu_cls, np.zeros((NCORES, pad), dtype=np.int64)], axis=1
    )
    wu = np.ascontiguousarray(
        wb[wu_cls_p.reshape(-1)]
        .reshape(NCORES, nwg, GW, KC, 128, DOUT)
        .transpose(0, 1, 4, 2, 3, 5)
    )

    nonzero_bias = bool(np.any(bias))
    nc = _build(run_lengths, nonzero_bias)

    in_maps = []
    for c in range(NCORES):
        m = {"xt": xt[c], "wu": wu[c]}
        if nonzero_bias:
            m["bias"] = np.ascontiguousarray(
                bias.reshape(1, DOUT).astype(np.float32)
            )
        in_maps.append(m)
    res = run_bass_kernel_spmd(
        nc, in_maps, core_ids=list(range(NCORES)), trace=_trace
    )
    LAST["results"] = res
    LAST["nruns"] = nruns

    # out[c][g, p, j, t, o] -> out[perm[c, g*G+j], n=(t,p), o]
    full = np.empty((B, NTOK, DOUT), dtype=np.float32)
    for c in range(NCORES):
        o = np.asarray(res.results[c]["out"])
        o = o.transpose(0, 2, 3, 1, 4).reshape(BLOC, NTOK, DOUT).astype(np.float32)
        full[perm[c]] = o
    return full
